# revision 17
# baseline (speedup 1.0000x reference)
"""ListMLE loss kernel for Trainium2 (8 NeuronCores, Bass/Tile).

loss = mean(logcumsumexp(outputs[t, labels[t]], axis=1) - outputs)

The per-row gather is done with per-partition local_scatter (GPSIMD
streams at ~line rate, unlike per-index ap_gather):
  host: counting-sort each row's labels within two position groups of
        2046 (ints only) -> run-start offsets OFF_g + sort perm PERM_g;
  device, per 128-row tile: E = exp(o) in bf16; per group g:
        T_g = local_scatter(E by OFF_g)       (E value at each run start)
        S_g = ttscan(mask*state + T_g)        (fill values through runs)
        G band = local_scatter(S_g by PERM_g) (back to original order;
                                               PERM is duplicate-free)
  the 4 tail positions (4092..4095) for ALL 8 tiles are gathered once
  per core by a single SWDGE indirect DMA (flat int32 element offsets
  into the DRAM outputs tensor), exp'ed once into a [128, 32] strip,
  and copied per-tile into G's tail band by the ACT engine;
  then C = cumsum(G), ln(C) accumulated, minus sum(outputs); the 8
  per-core [128,1] partials are summed on host (the all-reduce mean).

The tile loop is software-pipelined: dma+exp for tile t+1 issue before
tile t's scatter chain so ACT's Ln(t) never blocks exp(t+1).
"""

import numpy as np

import concourse.bacc as bacc
import concourse.bass as bass
import concourse.mybir as mybir
import concourse.tile as tile
import concourse.bass_isa as bass_isa
from concourse.bass_utils import run_bass_kernel_spmd

B, N = 8192, 4096
N_CORES = 8
ROWS = B // N_CORES          # 1024
TILES = ROWS // 128          # 8
GS = 2046                    # slots per sorted position-group
GROUPS = [(0, GS), (GS, GS)]
TAIL0 = 2 * GS               # 4092
NTAIL = N - TAIL0            # 4

_NC = None


def _local_scatter(gp, out_ap, data_ap, idxs_ap, num_elems, num_idxs):
    # like nc.gpsimd.local_scatter but allows any num_elems*32 <= 65472
    assert num_elems * 32 <= 65472 and num_elems % 2 == 0 and num_idxs % 2 == 0
    return gp.add_instruction(bass_isa.InstLocalScatter(
        name=f"I-{gp.bass.next_id()}",
        ins=[gp.lower_ap(data_ap, for_isa=True),
             gp.lower_ap(idxs_ap, for_isa=True)],
        outs=[gp.lower_ap(out_ap, for_isa=True)],
        _channels=128, _num_elems=num_elems, _num_idxs=num_idxs))


def _build(reps=1):
    nc = bacc.Bacc("TRN2", target_bir_lowering=False, debug=False,
                   num_devices=N_CORES)
    f32 = mybir.dt.float32
    bf16 = mybir.dt.bfloat16
    i16 = mybir.dt.int16
    i32 = mybir.dt.int32
    add = mybir.AluOpType.add

    O = nc.dram_tensor("outputs", [ROWS, N], f32, kind="ExternalInput").ap()
    OFFS = [nc.dram_tensor(f"off{g}", [ROWS, N], i16,
                           kind="ExternalInput").ap() for g in range(2)]
    PERMS = [nc.dram_tensor(f"perm{g}", [ROWS, GS], i16,
                            kind="ExternalInput").ap() for g in range(2)]
    AM = nc.dram_tensor("amask", [ROWS, TAIL0], bf16,
                        kind="ExternalInput").ap()
    TIDX = nc.dram_tensor("tailidx", [128, NTAIL * TILES], i32,
                          kind="ExternalInput").ap()
    OUT = nc.dram_tensor("out", [128, 1], f32, kind="ExternalOutput").ap()

    with tile.TileContext(nc) as tc:
        with tc.tile_pool(name="dma", bufs=2) as dpool, \
             tc.tile_pool(name="cmp", bufs=2) as cpool, \
             tc.tile_pool(name="cs", bufs=3) as cspool, \
             tc.tile_pool(name="sm", bufs=1) as spool:
            # per-tile partial sums land in their own strip columns (no
            # shared-accumulator dependency chain across tiles)
            nreps = reps * TILES
            lnstrip = spool.tile([128, nreps], f32, name="lnstrip")
            ostrip = spool.tile([128, nreps], f32, name="ostrip")

            state = {}

            def front(i, first=False):
                # dma + exp + osum for pass-tile i (runs ahead of the
                # previous tile's scatter chain)
                t = i % TILES
                r0 = 128 * t
                o = dpool.tile([128, N], f32, name="o", tag="o")
                nc.sync.dma_start(out=o[:], in_=O[r0:r0 + 128, :])
                offs = [dpool.tile([128, N], i16, name=f"offt{g}",
                                   tag=f"offt{g}") for g in range(2)]
                perms = [dpool.tile([128, GS], i16, name=f"permt{g}",
                                    tag=f"permt{g}") for g in range(2)]
                am = dpool.tile([128, TAIL0], bf16, name="am", tag="am")
                nc.sync.dma_start(out=offs[0][:], in_=OFFS[0][r0:r0 + 128, :])
                if first:
                    _tail_setup()
                nc.sync.dma_start(out=offs[1][:], in_=OFFS[1][r0:r0 + 128, :])
                nc.sync.dma_start(out=am[:], in_=AM[r0:r0 + 128, :])
                for g in range(2):
                    nc.sync.dma_start(out=perms[g][:],
                                      in_=PERMS[g][r0:r0 + 128, :])
                e = cpool.tile([128, N], bf16, name="e", tag="e")
                nc.scalar.activation(e[:], o[:],
                                     mybir.ActivationFunctionType.Exp)
                # sum(outputs) = sum(ln(e)) via ACT ln-accumulate; reading
                # bf16 e instead of f32 o halves the ACT port traffic
                trash = cpool.tile([128, N], bf16, name="trash", tag="lnt")
                nc.scalar.activation(trash[:], e[:],
                                     mybir.ActivationFunctionType.Ln,
                                     accum_out=ostrip[:, i:i + 1])
                state[i] = (e, offs, perms, am)

            tailbuf = {}

            def _tail_setup():
                # tail gather for all tiles: one SWDGE indirect DMA of the
                # 4*TILES tail elements per partition, then one exp strip
                tidx = spool.tile([128, NTAIL * TILES], i32, name="tidx")
                nc.sync.dma_start(out=tidx[:], in_=TIDX[:])
                traw = spool.tile([128, NTAIL * TILES], f32, name="traw")
                nc.gpsimd.indirect_dma_start(
                    out=traw[:], out_offset=None, in_=O,
                    in_offset=bass.IndirectOffsetOnAxis(ap=tidx[:], axis=1))
                texp = spool.tile([128, NTAIL * TILES], f32, name="texp")
                nc.scalar.activation(texp[:], traw[:],
                                     mybir.ActivationFunctionType.Exp)
                tailbuf["texp"] = texp

            def back_pool(i):
                # stage-1 scatters for pass-tile i; issued a stage early
                # so POOL never waits on tile i-1's fill scan. The fill
                # scan (host-precomputed run-start mask) is interleaved
                # between the two stage-1 scatters so its latency hides
                # under the second scatter and stage-2 starts immediately.
                t = i % TILES
                e, offs, perms, am = state.pop(i)
                G = cpool.tile([128, N], bf16, name="G", tag="G")
                # tail band: copy this tile's 4 pre-exp'ed values in
                nc.scalar.activation(G[:, TAIL0:N],
                                     tailbuf["texp"][:,
                                                     NTAIL * t:NTAIL * (t + 1)],
                                     mybir.ActivationFunctionType.Copy)

                T = cpool.tile([128, TAIL0], bf16, name="T", tag="T")
                S = cpool.tile([128, TAIL0], bf16, name="S", tag="S")
                for g, (st, sz) in enumerate(GROUPS):
                    _local_scatter(nc.gpsimd, T[:, st:st + sz], e[:],
                                   offs[g][:], num_elems=sz, num_idxs=N)
                # one fused fill over both bands: the host mask has 0 at
                # each band's slot 0, so the recurrence resets at the seam
                nc.vector.tensor_tensor_scan(
                    S[:], am[:], T[:], 0.0, mybir.AluOpType.mult, add)
                state[("b", i)] = (G, S, perms)

            def back_rest(i):
                # stage-2 scatters + cumsum for tile i (ln is deferred one
                # more iteration so ACT's in-order queue never stalls the
                # next tile's exp behind this tile's cumsum)
                G, S, perms = state.pop(("b", i))
                for g, (st, sz) in enumerate(GROUPS):
                    _local_scatter(nc.gpsimd, G[:, st:st + sz],
                                   S[:, st:st + sz], perms[g][:],
                                   num_elems=sz, num_idxs=sz)
                C = cspool.tile([128, N], bf16, name="C", tag="C")
                # data1 is ignored under op1=bypass; a stride-0 broadcast
                # keeps the rd1 SBUF port traffic off the shared port that
                # GPSIMD contends on
                nc.vector.tensor_tensor_scan(C[:], G[:],
                                             G[:, :1].to_broadcast([128, N]),
                                             0.0, add, mybir.AluOpType.bypass)
                state[("c", i)] = C

            def ln_late(i):
                C = state.pop(("c", i))
                lnt = cpool.tile([128, N], bf16, name="lnt2", tag="lnt")
                nc.scalar.activation(lnt[:], C[:],
                                     mybir.ActivationFunctionType.Ln,
                                     accum_out=lnstrip[:, i:i + 1])

            total = reps * TILES
            front(0, first=True)
            back_pool(0)
            for i in range(total):
                if i + 1 < total:
                    front(i + 1)
                    back_pool(i + 1)
                back_rest(i)
                if i - 1 >= 0:
                    ln_late(i - 1)
            ln_late(total - 1)

            lnred = spool.tile([128, 1], f32, name="lnred")
            nc.vector.tensor_reduce(lnred[:], lnstrip[:],
                                    axis=mybir.AxisListType.X, op=add)
            ored = spool.tile([128, 1], f32, name="ored")
            nc.vector.tensor_reduce(ored[:], ostrip[:],
                                    axis=mybir.AxisListType.X, op=add)
            comb = spool.tile([128, 1], f32, name="comb")
            nc.vector.tensor_tensor(out=comb[:], in0=lnred[:], in1=ored[:],
                                    op=mybir.AluOpType.subtract)
            nc.sync.dma_start(out=OUT[:], in_=comb[:])
    nc.compile()
    return nc


def _get_nc():
    global _NC
    if _NC is None:
        _NC = _build()
    return _NC


def _prep_inputs(outputs, labels):
    outputs = np.ascontiguousarray(np.asarray(outputs), dtype=np.float32)
    lab = np.asarray(labels).astype(np.int16)          # values in [0, 4096)
    # one radix argsort over the first 4092 cols; key = label | group<<12
    key = lab[:, :TAIL0].copy()
    key[:, GS:] += np.int16(1 << 12)
    si_full = np.argsort(key, axis=1, kind="stable")
    sk_full = np.sort(key, axis=1, kind="stable")

    import ml_dtypes
    offs, perms = [], []
    # fill mask in slot space: 0 at run starts (slot 0 or label change),
    # 1 elsewhere; S = ttscan(am*state + T) then forward-fills the runs
    amask = np.ones((B, TAIL0), dtype=ml_dtypes.bfloat16)
    for g, (st, sz) in enumerate(GROUPS):
        si = (si_full[:, st:st + sz] - st).astype(np.int16)
        SL = (sk_full[:, st:st + sz] - np.int16(g << 12)).astype(np.int16)
        off = np.full((B, N), -1, dtype=np.int16)
        # write slots in descending order so the run START wins
        slots = np.broadcast_to(
            np.arange(sz - 1, -1, -1, dtype=np.int16), (B, sz))
        np.put_along_axis(off, SL[:, ::-1].astype(np.int64), slots, axis=1)
        offs.append(off)
        perms.append(si)
        start = np.ones((B, sz), dtype=bool)
        start[:, 1:] = SL[:, 1:] != SL[:, :-1]
        amask[:, st:st + sz] = (~start).astype(ml_dtypes.bfloat16)

    # flat element offsets into the per-core [ROWS, N] outputs tensor for
    # the NTAIL tail labels of every tile: tailidx[p, NTAIL*t + c] =
    # (128*t + p)*N + labels[row, TAIL0 + c]
    tail = lab[:, TAIL0:].astype(np.int64)             # [B, NTAIL]
    tailidx_full = np.empty((N_CORES, 128, NTAIL * TILES), dtype=np.int32)
    rowbase = (np.arange(TILES)[None, :, None] * 128
               + np.arange(128)[:, None, None]) * N    # [128, TILES, 1]
    for c in range(N_CORES):
        tcore = tail[c * ROWS:(c + 1) * ROWS].reshape(TILES, 128, NTAIL)
        tcore = np.transpose(tcore, (1, 0, 2))         # [128, TILES, NTAIL]
        tailidx_full[c] = (tcore + rowbase).reshape(128, NTAIL * TILES)

    in_maps = []
    for c in range(N_CORES):
        sl = slice(c * ROWS, (c + 1) * ROWS)
        m = {"outputs": outputs[sl], "tailidx": tailidx_full[c],
             "amask": amask[sl]}
        for g in range(2):
            m[f"off{g}"] = offs[g][sl]
            m[f"perm{g}"] = perms[g][sl]
        in_maps.append(m)
    return in_maps


def kernel(outputs, labels):
    nc = _get_nc()
    in_maps = _prep_inputs(outputs, labels)
    res = run_bass_kernel_spmd(nc, in_maps, core_ids=list(range(N_CORES)))
    total = sum(float(r["out"].sum()) for r in res.results)
    return np.float32(total / (B * N))
